# revision 30
# baseline (speedup 1.0000x reference)
"""Bradley-Terry loss kernel for Trainium2 — Chebyshev/PE design, v3 (fp8 DoubleRow).

loss = sum_{i!=j} W[i,j] * softplus(b_j - b_i)
     = sum_{m,l} A[m,l] * z[m,l] - ln2 * trace(W),
  z[m,l] = sum_ij W_ij T_m(x_i) T_l(x_j),  x = (b - c)/h in [-1,1]

Per core TensorE computes Y[m, j] = sum_{i in shard} W[i, j] T_m(x_i)
with the Chebyshev basis as stationary.  v3 uses fp8e4m3 operands with
perf_mode=DoubleRow: each matmul contracts K=256 (two 128-row tiles
paired on the k-subtile axis), halving PE instruction count and cycles
so the (HAM-throttled, 1.2GHz-cold) PE can never pace the DMA stream.
fp8 rounding of W and the basis costs ~2e-4 relative error (budget 2e-2).

The column contraction stays on-device: host precomputes
B[m, j] = sum_l A[m, l] T_l(x_j)  (= A @ C^T, bf16 [64, N]), and VectorE
dots each chunk's PSUM [64, 2048] against B with accum_out, so per-core
output is a 1KB partial vector instead of 4MB of Y.  Consts ride the
Activation HWDGE queue; the sync queue is a pure W read stream.
"""

import numpy as np
import ml_dtypes

import concourse.bacc as bacc
import concourse.bass as bass
import concourse.mybir as mybir
from concourse import tile
from concourse.bass_utils import run_bass_kernel_spmd

N = 8192
NCORES = 8
R = N // NCORES            # 1024 rows per core
P = 128                    # SBUF partitions
TROWS = R // P             # 8 row-tiles per core
TPAIR = TROWS // 2         # 4 DoubleRow tile-pairs (K=256 each)
CHALF = 2048               # max column group per PSUM generation
# narrower trailing chunks shrink the exposed tail after the last W byte
WIDTHS = (2048, 2048, 2048, 1536, 512)
COL0 = (0, 2048, 4096, 6144, 7680)
NCHUNK = len(WIDTHS)
SLAB = 512                 # PSUM bank free size (fp32)
NSLABS = tuple(wd // SLAB for wd in WIDTHS)
SLAB0 = tuple(sum(NSLABS[:i]) for i in range(NCHUNK))
NACC = sum(NSLABS)         # one accumulator column per slab dot
DEG = 63
M1 = DEG + 1               # 64 chebyshev coefficients
_LN2 = float(np.log(2.0))

_cached_nc = None


def _cheb_vals(x, deg):
    out = np.empty((len(x), deg + 1), dtype=np.float64)
    out[:, 0] = 1.0
    if deg >= 1:
        out[:, 1] = x
    for k in range(2, deg + 1):
        out[:, k] = 2 * x * out[:, k - 1] - out[:, k - 2]
    return out


def _cheb2d_coeffs(f, deg):
    n = deg + 1
    theta = (np.arange(n) + 0.5) * np.pi / n
    pts = np.cos(theta)
    F = f(pts[:, None], pts[None, :])
    Tm = np.cos(np.outer(np.arange(n), theta))
    A = (2.0 / n) * Tm @ F @ ((2.0 / n) * Tm).T
    A[0, :] /= 2
    A[:, 0] /= 2
    return A


def _build():
    nc = bacc.Bacc(
        "TRN2",
        target_bir_lowering=False,
        debug=False,
        enable_asserts=False,
        num_devices=NCORES,
    )
    f32 = mybir.dt.float32
    bf16 = mybir.dt.bfloat16
    f8 = mybir.dt.float8e4
    w = nc.dram_tensor("w", [R, N], f32, kind="ExternalInput")
    # [p, tp, ks, m] = T_m(x_{tp*256 + ks*128 + p}) in fp8
    crows = nc.dram_tensor("crows", [P, TPAIR * 2 * M1], f8, kind="ExternalInput")
    bmat = nc.dram_tensor("bmat", [M1, N], f8, kind="ExternalInput")
    acc = nc.dram_tensor("acc", [M1, NACC], f32, kind="ExternalOutput")

    with tile.TileContext(nc) as tc:
        with (
            tc.tile_pool(name="consts", bufs=1) as consts,
            tc.tile_pool(name="wpool", bufs=4) as wpool,
            tc.tile_pool(name="wbpool", bufs=8) as wbpool,
            tc.tile_pool(name="scrpool", bufs=2) as scrpool,
            tc.tile_pool(name="psum", bufs=2, space="PSUM") as pspool,
        ):
            crows_sb = consts.tile([P, TPAIR * 2 * M1], f8)
            nc.scalar.dma_start(crows_sb[:], crows.ap())
            bmat_sb = consts.tile([M1, N], f8)
            # fp8 B halves const traffic; full rows are 8KB descriptors
            # (16KB descriptors measured at half the per-byte rate)
            nc.scalar.dma_start(bmat_sb[:], bmat.ap())
            acc_sb = consts.tile([M1, NACC], f32)
            crows_v = crows_sb.rearrange("p (tp ks m) -> p tp ks m", tp=TPAIR, ks=2)

            # one dot per 512-col slab (not per chunk): a full-chunk dot is
            # a ~1.7us DVE block that, deferred into a narrow next chunk,
            # stalls its casts on the strict-FIFO DVE; slab dots slot into
            # the idle gaps between tile-pair cast groups
            def emit_dot(ps_ch, ci, s):
                c0 = COL0[ci] + s * SLAB
                scr = scrpool.tile([M1, SLAB], f32, tag="scr")
                nc.vector.scalar_tensor_tensor(
                    out=scr[:],
                    in0=ps_ch[:, s * SLAB : (s + 1) * SLAB],
                    scalar=0.0,
                    in1=bmat_sb[:, c0 : c0 + SLAB],
                    op0=mybir.AluOpType.bypass,
                    op1=mybir.AluOpType.mult,
                    accum_out=acc_sb[:, SLAB0[ci] + s : SLAB0[ci] + s + 1],
                )

            pending = None
            for ci in range(NCHUNK):
                c0, cw = COL0[ci], WIDTHS[ci]
                nslab = cw // SLAB
                ps = pspool.tile([M1, CHALF], f32, tag="ps", name=f"ps_{ci}")
                for tp in range(TPAIR):
                    # one DMA per 256-row pair, already in DoubleRow [p, ks, j]
                    # layout: descriptor per (p, ks) row segment, still 8KB each
                    wt = wpool.tile([P, 2, CHALF], f32, tag="w")
                    nc.sync.dma_start(
                        wt[:, :, :cw],
                        w.ap()[tp * 2 * P : (tp + 1) * 2 * P, c0 : c0 + cw]
                        .rearrange("(ks p) j -> p ks j", ks=2),
                    )
                    lhsT = crows_v[:, tp, :, :]
                    for s in range(nslab):
                        # alternate fp8 casts between DVE and the otherwise
                        # idle Activation engine: halves DVE occupancy and
                        # de-serializes the cast->MM chain at the tail
                        wb = wbpool.tile([P, 2, SLAB], f8, tag="wb")
                        src = wt[:, :, s * SLAB : (s + 1) * SLAB]
                        if s % 2 == 0:
                            nc.vector.tensor_copy(wb[:], src)
                        else:
                            nc.scalar.copy(wb[:], src)
                        nc.tensor.matmul(
                            ps[:, s * SLAB : (s + 1) * SLAB],
                            lhsT,
                            wb[:],
                            start=(tp == 0),
                            stop=(tp == TPAIR - 1),
                            perf_mode=mybir.MatmulPerfMode.DoubleRow,
                        )
                    # deferred slab-dots of the previous chunk, one per
                    # tile-pair slot (its psum group closed last chunk)
                    if pending is not None and tp < NSLABS[ci - 1]:
                        emit_dot(pending, ci - 1, tp)
                pending = ps
            for s in range(NSLABS[NCHUNK - 1]):
                emit_dot(pending, NCHUNK - 1, s)
            nc.scalar.dma_start(acc.ap(), acc_sb[:])

    nc.compile()
    return nc


def _get_nc():
    global _cached_nc
    if _cached_nc is None:
        _cached_nc = _build()
    return _cached_nc


def kernel(win_matrix, betas, _trace=False):
    win_matrix = np.asarray(win_matrix, dtype=np.float32)
    betas = np.asarray(betas, dtype=np.float32)
    nc = _get_nc()

    b64 = betas.astype(np.float64)
    lo, hi = float(b64.min()), float(b64.max())
    c = 0.5 * (lo + hi)
    h = max(0.5 * (hi - lo) * 1.000001, 1e-12)
    x = (b64 - c) / h
    A = _cheb2d_coeffs(lambda X, Y: np.logaddexp(0.0, h * (Y - X)), DEG)
    C = _cheb_vals(x, DEG)                       # [N, 64] f64
    C8 = C.astype(ml_dtypes.float8_e4m3fn)

    # B[m, j] = sum_l A[m, l] T_l(x_j)
    B = A @ C.T                                  # [64, N] f64
    bmat_np = np.ascontiguousarray(B.astype(ml_dtypes.float8_e4m3fn))

    in_maps = []
    for cc in range(NCORES):
        rows = slice(cc * R, (cc + 1) * R)
        # [p, tp, ks, m] packing of the fp8 basis for DoubleRow K=256
        crows_np = np.ascontiguousarray(
            C8[rows].reshape(TPAIR, 2, P, M1).transpose(2, 0, 1, 3).reshape(P, -1)
        )
        in_maps.append(
            {
                "w": np.ascontiguousarray(win_matrix[rows]),
                "crows": crows_np,
                "bmat": bmat_np,
            }
        )
    res = run_bass_kernel_spmd(
        nc, in_maps, core_ids=list(range(NCORES)), trace=_trace
    )

    total = 0.0
    for cc in range(NCORES):
        total += float(res.results[cc]["acc"].astype(np.float64).sum())
    total -= _LN2 * float(np.trace(win_matrix.astype(np.float64)))
    if _trace:
        kernel.last_results = res
    return np.array(total, dtype=np.float32)


# revision 31
# speedup vs baseline: 1.1376x; 1.1376x over previous
"""Bradley-Terry loss kernel for Trainium2 — Chebyshev/PE design, v3 (fp8 DoubleRow).

loss = sum_{i!=j} W[i,j] * softplus(b_j - b_i)
     = sum_{m,l} A[m,l] * z[m,l] - ln2 * trace(W),
  z[m,l] = sum_ij W_ij T_m(x_i) T_l(x_j),  x = (b - c)/h in [-1,1]

Per core TensorE computes Y[m, j] = sum_{i in shard} W[i, j] T_m(x_i)
with the Chebyshev basis as stationary.  v3 uses fp8e4m3 operands with
perf_mode=DoubleRow: each matmul contracts K=256 (two 128-row tiles
paired on the k-subtile axis), halving PE instruction count and cycles
so the (HAM-throttled, 1.2GHz-cold) PE can never pace the DMA stream.
fp8 rounding of W and the basis costs ~2e-4 relative error (budget 2e-2).

The column contraction stays on-device: host precomputes
B[m, j] = sum_l A[m, l] T_l(x_j)  (= A @ C^T, bf16 [64, N]), and VectorE
dots each chunk's PSUM [64, 2048] against B with accum_out, so per-core
output is a 1KB partial vector instead of 4MB of Y.  Consts ride the
Activation HWDGE queue; the sync queue is a pure W read stream.
"""

import numpy as np
import ml_dtypes

import concourse.bacc as bacc
import concourse.bass as bass
import concourse.mybir as mybir
from concourse import tile
from concourse.bass_utils import run_bass_kernel_spmd

N = 8192
NCORES = 8
R = N // NCORES            # 1024 rows per core
P = 128                    # SBUF partitions
TROWS = R // P             # 8 row-tiles per core
TPAIR = TROWS // 2         # 4 DoubleRow tile-pairs (K=256 each)
CHALF = 2048               # max column group per PSUM generation
# narrower trailing chunks shrink the exposed tail after the last W byte
WIDTHS = (2048, 2048, 2048, 1536, 512)
COL0 = (0, 2048, 4096, 6144, 7680)
NCHUNK = len(WIDTHS)
SLAB = 512                 # PSUM bank free size (fp32)
NSLABS = tuple(wd // SLAB for wd in WIDTHS)
SLAB0 = tuple(sum(NSLABS[:i]) for i in range(NCHUNK))
NACC = sum(NSLABS)         # one accumulator column per slab dot
DEG = 63
M1 = DEG + 1               # 64 chebyshev coefficients
_LN2 = float(np.log(2.0))

_cached_nc = None


def _cheb_vals(x, deg):
    out = np.empty((len(x), deg + 1), dtype=np.float64)
    out[:, 0] = 1.0
    if deg >= 1:
        out[:, 1] = x
    for k in range(2, deg + 1):
        out[:, k] = 2 * x * out[:, k - 1] - out[:, k - 2]
    return out


def _cheb2d_coeffs(f, deg):
    n = deg + 1
    theta = (np.arange(n) + 0.5) * np.pi / n
    pts = np.cos(theta)
    F = f(pts[:, None], pts[None, :])
    Tm = np.cos(np.outer(np.arange(n), theta))
    A = (2.0 / n) * Tm @ F @ ((2.0 / n) * Tm).T
    A[0, :] /= 2
    A[:, 0] /= 2
    return A


def _build():
    nc = bacc.Bacc(
        "TRN2",
        target_bir_lowering=False,
        debug=False,
        enable_asserts=False,
        num_devices=NCORES,
    )
    f32 = mybir.dt.float32
    bf16 = mybir.dt.bfloat16
    f8 = mybir.dt.float8e4
    w = nc.dram_tensor("w", [R, N], f32, kind="ExternalInput")
    # [p, tp, ks, m] = T_m(x_{tp*256 + ks*128 + p}) in fp8
    crows = nc.dram_tensor("crows", [P, TPAIR * 2 * M1], f8, kind="ExternalInput")
    bmat = nc.dram_tensor("bmat", [M1, N], f8, kind="ExternalInput")
    acc = nc.dram_tensor("acc", [M1, NACC], f32, kind="ExternalOutput")

    with tile.TileContext(nc) as tc:
        with (
            tc.tile_pool(name="consts", bufs=1) as consts,
            tc.tile_pool(name="wpool", bufs=4) as wpool,
            tc.tile_pool(name="wbpool", bufs=8) as wbpool,
            tc.tile_pool(name="scrpool", bufs=2) as scrpool,
            tc.tile_pool(name="psum", bufs=2, space="PSUM") as pspool,
        ):
            crows_sb = consts.tile([P, TPAIR * 2 * M1], f8)
            nc.scalar.dma_start(crows_sb[:], crows.ap())
            bmat_sb = consts.tile([M1, N], f8)
            # fp8 B halves const traffic; full rows are 8KB descriptors
            # (16KB descriptors measured at half the per-byte rate)
            nc.scalar.dma_start(bmat_sb[:], bmat.ap())
            acc_sb = consts.tile([M1, NACC], f32)
            crows_v = crows_sb.rearrange("p (tp ks m) -> p tp ks m", tp=TPAIR, ks=2)

            # one dot per 512-col slab (not per chunk): a full-chunk dot is
            # a ~1.7us DVE block that, deferred into a narrow next chunk,
            # stalls its casts on the strict-FIFO DVE; slab dots slot into
            # the idle gaps between tile-pair cast groups
            def emit_dot(ps_ch, ci, s):
                c0 = COL0[ci] + s * SLAB
                scr = scrpool.tile([M1, SLAB], f32, tag="scr")
                nc.vector.scalar_tensor_tensor(
                    out=scr[:],
                    in0=ps_ch[:, s * SLAB : (s + 1) * SLAB],
                    scalar=0.0,
                    in1=bmat_sb[:, c0 : c0 + SLAB],
                    op0=mybir.AluOpType.bypass,
                    op1=mybir.AluOpType.mult,
                    accum_out=acc_sb[:, SLAB0[ci] + s : SLAB0[ci] + s + 1],
                )

            pending = None
            for ci in range(NCHUNK):
                c0, cw = COL0[ci], WIDTHS[ci]
                nslab = cw // SLAB
                ps = pspool.tile([M1, CHALF], f32, tag="ps", name=f"ps_{ci}")
                for tp in range(TPAIR):
                    # one DMA per 256-row pair, already in DoubleRow [p, ks, j]
                    # layout: descriptor per (p, ks) row segment, still 8KB each
                    wt = wpool.tile([P, 2, CHALF], f32, tag="w")
                    nc.sync.dma_start(
                        wt[:, :, :cw],
                        w.ap()[tp * 2 * P : (tp + 1) * 2 * P, c0 : c0 + cw]
                        .rearrange("(ks p) j -> p ks j", ks=2),
                    )
                    lhsT = crows_v[:, tp, :, :]
                    for s in range(nslab):
                        wb = wbpool.tile([P, 2, SLAB], f8, tag="wb")
                        nc.vector.tensor_copy(
                            wb[:], wt[:, :, s * SLAB : (s + 1) * SLAB]
                        )
                        nc.tensor.matmul(
                            ps[:, s * SLAB : (s + 1) * SLAB],
                            lhsT,
                            wb[:],
                            start=(tp == 0),
                            stop=(tp == TPAIR - 1),
                            perf_mode=mybir.MatmulPerfMode.DoubleRow,
                        )
                    # deferred slab-dots of the previous chunk, one per
                    # tile-pair slot (its psum group closed last chunk)
                    if pending is not None and tp < NSLABS[ci - 1]:
                        emit_dot(pending, ci - 1, tp)
                pending = ps
            for s in range(NSLABS[NCHUNK - 1]):
                emit_dot(pending, NCHUNK - 1, s)
            nc.scalar.dma_start(acc.ap(), acc_sb[:])

    nc.compile()
    return nc


def _get_nc():
    global _cached_nc
    if _cached_nc is None:
        _cached_nc = _build()
    return _cached_nc


def kernel(win_matrix, betas, _trace=False):
    win_matrix = np.asarray(win_matrix, dtype=np.float32)
    betas = np.asarray(betas, dtype=np.float32)
    nc = _get_nc()

    b64 = betas.astype(np.float64)
    lo, hi = float(b64.min()), float(b64.max())
    c = 0.5 * (lo + hi)
    h = max(0.5 * (hi - lo) * 1.000001, 1e-12)
    x = (b64 - c) / h
    A = _cheb2d_coeffs(lambda X, Y: np.logaddexp(0.0, h * (Y - X)), DEG)
    C = _cheb_vals(x, DEG)                       # [N, 64] f64
    C8 = C.astype(ml_dtypes.float8_e4m3fn)

    # B[m, j] = sum_l A[m, l] T_l(x_j)
    B = A @ C.T                                  # [64, N] f64
    bmat_np = np.ascontiguousarray(B.astype(ml_dtypes.float8_e4m3fn))

    in_maps = []
    for cc in range(NCORES):
        rows = slice(cc * R, (cc + 1) * R)
        # [p, tp, ks, m] packing of the fp8 basis for DoubleRow K=256
        crows_np = np.ascontiguousarray(
            C8[rows].reshape(TPAIR, 2, P, M1).transpose(2, 0, 1, 3).reshape(P, -1)
        )
        in_maps.append(
            {
                "w": np.ascontiguousarray(win_matrix[rows]),
                "crows": crows_np,
                "bmat": bmat_np,
            }
        )
    res = run_bass_kernel_spmd(
        nc, in_maps, core_ids=list(range(NCORES)), trace=_trace
    )

    total = 0.0
    for cc in range(NCORES):
        total += float(res.results[cc]["acc"].astype(np.float64).sum())
    total -= _LN2 * float(np.trace(win_matrix.astype(np.float64)))
    if _trace:
        kernel.last_results = res
    return np.array(total, dtype=np.float32)
